# revision 1
# baseline (speedup 1.0000x reference)
"""Masked multi-head attention (sparse_attention) Trainium2 Bass kernel.

Data-parallel over batch: B=8 batch elements, one per NeuronCore.
Per-core computation for batch element b (all shapes hardcoded):
  x [1024,768], adj [1024,1024], Wq/Wk/Wv [768,768], bq/bk/bv [768], beta []
  q = x@Wq+bq; k = x@Wk+bk; v = x@Wv+bv      (12 heads of 64)
  S = q k^T / 8 + beta*adj ; masked where adj<=0 ; P = softmax(S)
  out = P v  -> [1024, 768]

Kernel strategy (per core):
  - X^T via PE transposes; Q^T,K^T = W^T-chunk matmuls (float32r, 1cyc/row)
    stored bf16 as [768,1024] so head-pair 2c,2c+1 sits in partition halves
    of tile c.  V stored bf16 as [1024, 12*65] with a ones column per head
    (the ones column makes the PV matmul emit softmax row-sums for free).
  - m^T = (adjT>0)*exp(beta*adjT) once per batch (shared by all 12 heads);
    then per head P^T = m^T * exp(S^T/8) with S^T = K_h @ Q_h^T (K=64 MMs).
    No max-subtraction needed: logits are O(1) for this problem.
  - out^T[65,512] = [V_h|1]^T @ P^T  (N=512 matmuls), PE-transpose back,
    reciprocal of column 64 (row-sum) scales the head output.
"""

import sys

import numpy as np

try:
    import concourse.bass as bass
except ImportError:  # container default location
    sys.path.insert(0, "/opt/trn_rl_repo")
    import concourse.bass as bass

from contextlib import ExitStack

import concourse.bacc as bacc
import concourse.mybir as mybir
import concourse.tile as tile
from concourse.bass_utils import run_bass_kernel_spmd
from concourse.masks import make_identity

B, N, D, H, HD = 8, 1024, 768, 12, 64
P = 128
NT = N // P  # 8 row chunks
DT = D // P  # 6 feature chunks
NH = 512  # free-dim tile for matmuls
HD1 = HD + 1  # head dim + ones column

F32 = mybir.dt.float32
F32R = mybir.dt.float32r
BF16 = mybir.dt.bfloat16
AF = mybir.ActivationFunctionType
ALU = mybir.AluOpType


def _emit(tc, ctx, x_d, adj_d, w_d, b_d, beta_d, out_d):
    nc = tc.nc

    const = ctx.enter_context(tc.tile_pool(name="const", bufs=1))
    ident = const.tile([P, P], F32, tag="ident")
    make_identity(nc, ident)
    identb = const.tile([P, P], BF16, tag="identb")
    make_identity(nc, identb)
    def bcast(ap, n_part):
        return bass.AP(tensor=ap.tensor, offset=ap.offset, ap=[[0, n_part]] + list(ap.ap))

    beta_sb = const.tile([P, 1], F32, tag="beta")
    nc.gpsimd.dma_start(out=beta_sb, in_=bcast(beta_d[0], P))
    bq_sb = const.tile([P, DT], F32, tag="bq")
    nc.gpsimd.dma_start(out=bq_sb, in_=b_d["bq"].rearrange("(c p) -> p c", p=P))
    bk_sb = const.tile([P, DT], F32, tag="bk")
    nc.gpsimd.dma_start(out=bk_sb, in_=b_d["bk"].rearrange("(c p) -> p c", p=P))
    bv_bc = const.tile([P, D], F32, tag="bv")
    nc.gpsimd.dma_start(out=bv_bc, in_=bcast(b_d["bv"], P))

    # Persistent tensors (live across phases)
    pers = ctx.enter_context(tc.tile_pool(name="pers", bufs=1))
    qt = [pers.tile([P, N], BF16, tag=f"qt{c}", name=f"qt{c}") for c in range(DT)]
    kt = [pers.tile([P, N], BF16, tag=f"kt{c}", name=f"kt{c}") for c in range(DT)]
    v_sb = [pers.tile([P, H * HD1], BF16, tag=f"v{i}", name=f"v{i}") for i in range(NT)]
    m_sb = [pers.tile([P, N], BF16, tag=f"m{k}", name=f"m{k}") for k in range(NT)]
    out_sb = [pers.tile([P, D], F32, tag=f"os{i}", name=f"os{i}") for i in range(NT)]

    # ---------------- Phase 1-3: X^T, projections (bf16), mask ----------------
    # One PSUM scope: pstr(2) + psmm(4) + psadj(2) = 8 banks, no cross-phase reuse.
    with tc.tile_pool(name="xw", bufs=1) as xw, \
         tc.tile_pool(name="pstr", space="PSUM", bufs=2) as pstr, \
         tc.tile_pool(name="psmm", space="PSUM", bufs=4) as psmm, \
         tc.tile_pool(name="psadj", space="PSUM", bufs=2) as psadj, \
         tc.tile_pool(name="etp", bufs=4) as etp:
        x_sb = [xw.tile([P, D], F32, tag=f"x{i}", name=f"x{i}") for i in range(NT)]
        for i in range(NT):
            nc.sync.dma_start(out=x_sb[i], in_=x_d[i * P:(i + 1) * P, :])

        # W: f32 -> bf16 cast in-flight (SWDGE casting DMA)
        w_sb = {}
        for wname in ("wq", "wk", "wv"):
            w_sb[wname] = [
                xw.tile([P, D], BF16, tag=f"{wname}{c}", name=f"{wname}{c}")
                for c in range(DT)
            ]
            for c in range(DT):
                nc.gpsimd.dma_start(
                    out=w_sb[wname][c], in_=w_d[wname][c * P:(c + 1) * P, :]
                )

        adj_sb = [xw.tile([P, N], F32, tag=f"adj{i}", name=f"adj{i}") for i in range(NT)]
        for i in range(NT):
            nc.sync.dma_start(out=adj_sb[i], in_=adj_d[i * P:(i + 1) * P, :])

        xt = [xw.tile([P, N], BF16, tag=f"xt{c}", name=f"xt{c}") for c in range(DT)]
        for c in range(DT):
            for i in range(NT):
                tp = pstr.tile([P, P], F32, tag="tp", name="tp")
                nc.tensor.transpose(tp, x_sb[i][:, c * P:(c + 1) * P], ident)
                nc.scalar.copy(xt[c][:, i * P:(i + 1) * P], tp)

        # Q^T, K^T: out[d_out, n] accumulated over d_in chunks; bias per-partition
        for wname, dst, bias_sb in (("wq", qt, bq_sb), ("wk", kt, bk_sb)):
            for c in range(DT):
                for qh in range(2):
                    ps = psmm.tile([P, NH], F32, tag="mm", name="mm")
                    for kc in range(DT):
                        nc.tensor.matmul(
                            ps,
                            lhsT=w_sb[wname][kc][:, c * P:(c + 1) * P],
                            rhs=xt[kc][:, qh * NH:(qh + 1) * NH],
                            start=(kc == 0),
                            stop=(kc == DT - 1),
                        )
                    nc.vector.tensor_scalar_add(
                        dst[c][:, qh * NH:(qh + 1) * NH], ps, bias_sb[:, c:c + 1]
                    )

        # V projection interleaved with mask computation (keeps the PE stream
        # mixed so there is no solid block of low-rate transposes at the
        # proj->attention boundary).
        def emit_v(i):
            for s, w in ((0, NH), (NH, D - NH)):
                ps = psmm.tile([P, w], F32, tag="mm", name="mm")
                for kc in range(DT):
                    nc.tensor.matmul(
                        ps,
                        lhsT=xt[kc][:, i * P:(i + 1) * P],
                        rhs=w_sb["wv"][kc][:, s:s + w],
                        start=(kc == 0),
                        stop=(kc == DT - 1),
                    )
                nh = w // HD
                h0 = s // HD
                dst3 = v_sb[i].rearrange("p (h j) -> p h j", j=HD1)[:, h0:h0 + nh, 0:HD]
                src3 = ps.rearrange("p (h j) -> p h j", j=HD)
                bias3 = bv_bc[:, s:s + w].rearrange("p (h j) -> p h j", j=HD)
                nc.vector.tensor_add(dst3, src3, bias3)
            ones3 = v_sb[i].rearrange("p (h j) -> p h j", j=HD1)[:, :, HD:HD1]
            nc.vector.memset(ones3, 1.0)

        def emit_mask(k):
            # m^T[k] = (adjT>0)*exp(beta*adjT)
            for j in range(0, NT, 4):
                aps = psadj.tile([P, 4 * P], F32, tag="aps", name="aps")
                for bb in range(4):
                    nc.tensor.transpose(
                        aps[:, bb * P:(bb + 1) * P],
                        adj_sb[j + bb][:, k * P:(k + 1) * P],
                        ident,
                    )
                e = etp.tile([P, 4 * P], BF16, tag="e", name="e")
                nc.scalar.activation(e, aps, AF.Exp, scale=beta_sb[:, 0:1])
                nc.vector.scalar_tensor_tensor(
                    out=m_sb[k][:, j * P:(j + 4) * P],
                    in0=aps,
                    scalar=0.0,
                    in1=e,
                    op0=ALU.is_gt,
                    op1=ALU.mult,
                )

        for i in range(NT):
            emit_v(i)
        for k in range(NT):
            emit_mask(k)

    # ---------------- Phase 4: attention, software-pipelined across heads ----------------
    # PE program order: S(0), S(1), PV(0), S(2), PV(1), ... so the PE always has
    # a head of runway while ACT/DVE chew exp/mask of the previous head.
    ot_sb = [None] * H
    p_tiles = [None] * H

    with tc.tile_pool(name="pp", bufs=2) as pp, \
         tc.tile_pool(name="etq", bufs=3) as etq, \
         tc.tile_pool(name="otp", bufs=1) as otp:
      with tc.tile_pool(name="pss", space="PSUM", bufs=3) as pss, \
           tc.tile_pool(name="pso", space="PSUM", bufs=2) as pso:

        def emit_s_pair(c):
            # heads 2c (kt/qt rows 0:64) and 2c+1 (rows 64:128); the K=64
            # matmuls of the two heads run concurrently in disjoint row groups
            # of the PE array (tile_position row packing).  One psum bank per
            # matmul, separate tags per sub-head so paired matmuls become
            # ready together and stay adjacent in the PE queue.
            for sub in range(2):
                h = 2 * c + sub
                p_tiles[h] = [
                    pp.tile([P, N], BF16, tag=f"p{sub}_{k}", name=f"p{sub}_{k}")
                    for k in range(NT)
                ]
            for k in range(NT):
                sps = [pss.tile([P, N], F32, tag="s", name=f"s{sub}") for sub in range(2)]
                for qh in range(2):
                    for sub in range(2):
                        r0 = sub * HD
                        nc.tensor.matmul(
                            sps[sub][:, qh * NH:(qh + 1) * NH],
                            lhsT=kt[c][r0:r0 + HD, k * P:(k + 1) * P],
                            rhs=qt[c][r0:r0 + HD, qh * NH:(qh + 1) * NH],
                            start=True,
                            stop=True,
                            tile_position=(r0, 0),
                        )
                for sub in range(2):
                    e = etq.tile([P, N], BF16, tag="et", name="et")
                    nc.scalar.activation(e, sps[sub], AF.Exp, scale=0.125)
                    nc.vector.tensor_mul(p_tiles[2 * c + sub][k], e, m_sb[k])

        def emit_pv(h):
            p_t = p_tiles[h]
            ot = otp.tile([HD1, N], F32, tag=f"ot{h}", name=f"ot{h}")
            ot_sb[h] = ot
            for qh in range(2):
                ops = pso.tile([HD1, NH], F32, tag="ov", name="ov")
                for k in range(NT):
                    nc.tensor.matmul(
                        ops,
                        lhsT=v_sb[k][:, h * HD1:(h + 1) * HD1],
                        rhs=p_t[k][:, qh * NH:(qh + 1) * NH],
                        start=(k == 0),
                        stop=(k == NT - 1),
                    )
                nc.vector.tensor_copy(ot[:, qh * NH:(qh + 1) * NH], ops)

        for c in range(H // 2):
            emit_s_pair(c)
            if c >= 1:
                emit_pv(2 * c - 2)
                emit_pv(2 * c - 1)
        emit_pv(H - 2)
        emit_pv(H - 1)

      # ---- finalize: per q-chunk, transpose 6 heads at a time, scale by 1/rowsum
      with tc.tile_pool(name="fin", bufs=4) as fin, \
         tc.tile_pool(name="psf", space="PSUM", bufs=2) as psf:
          HG = H // 2  # 6 heads per psum tile ([128, 390] fits one bank)
          for qc in range(NT):
              for half in range(2):
                  fp = psf.tile([P, HG * HD1], F32, tag="fp", name="fp")
                  for hh in range(HG):
                      h = half * HG + hh
                      nc.tensor.transpose(
                          fp[:, hh * HD1:(hh + 1) * HD1],
                          ot_sb[h][:, qc * P:(qc + 1) * P],
                          ident[0:HD1, 0:HD1],
                      )
                  fp3 = fp.rearrange("p (h j) -> p h j", j=HD1)
                  rec = fin.tile([P, HG], F32, tag="rec", name="rec")
                  nc.vector.reciprocal(rec, fp3[:, :, HD:HD1].squeeze(-1))
                  rec_b = bass.AP(
                      tensor=rec.tensor,
                      offset=rec.offset,
                      ap=list(rec.ap) + [[0, HD]],
                  )
                  out3 = out_sb[qc].rearrange("p (h j) -> p h j", j=HD)
                  nc.vector.tensor_mul(
                      out3[:, half * HG:(half + 1) * HG, :],
                      fp3[:, :, 0:HD],
                      rec_b,
                  )
              nc.sync.dma_start(out=out_d[qc * P:(qc + 1) * P, :], in_=out_sb[qc])


def build_nc():
    nc = bacc.Bacc("TRN2", target_bir_lowering=False, debug=False, num_devices=B)
    x_d = nc.dram_tensor("x", [N, D], F32, kind="ExternalInput").ap()
    adj_d = nc.dram_tensor("adj", [N, N], F32, kind="ExternalInput").ap()
    w_d = {
        "wq": nc.dram_tensor("wq", [D, D], F32, kind="ExternalInput").ap(),
        "wk": nc.dram_tensor("wk", [D, D], F32, kind="ExternalInput").ap(),
        "wv": nc.dram_tensor("wv", [D, D], F32, kind="ExternalInput").ap(),
    }
    b_d = {
        "bq": nc.dram_tensor("bq", [D], F32, kind="ExternalInput").ap(),
        "bk": nc.dram_tensor("bk", [D], F32, kind="ExternalInput").ap(),
        "bv": nc.dram_tensor("bv", [D], F32, kind="ExternalInput").ap(),
    }
    beta_d = nc.dram_tensor("beta", [1, 1], F32, kind="ExternalInput").ap()
    out_d = nc.dram_tensor("out", [N, D], F32, kind="ExternalOutput").ap()
    with tile.TileContext(nc) as tc, ExitStack() as ctx:
        _emit(tc, ctx, x_d, adj_d, w_d, b_d, beta_d, out_d)
    nc.compile()
    return nc


_CACHE = {}


def _get_nc():
    if "nc" not in _CACHE:
        _CACHE["nc"] = build_nc()
    return _CACHE["nc"]


def make_in_maps(input_graph, adj, Wq, bq, Wk, bk, Wv, bv, beta):
    f = lambda a: np.ascontiguousarray(np.asarray(a), dtype=np.float32)
    wq, wk, wv = f(Wq), f(Wk), f(Wv)
    bqa, bka, bva = f(bq), f(bk), f(bv)
    beta_a = f(beta).reshape(1, 1)
    ig, ad = f(input_graph), f(adj)
    return [
        {
            "x": ig[b], "adj": ad[b],
            "wq": wq, "wk": wk, "wv": wv,
            "bq": bqa, "bk": bka, "bv": bva,
            "beta": beta_a,
        }
        for b in range(B)
    ]


def run_hw(in_maps, **kwargs):
    nc = _get_nc()
    return run_bass_kernel_spmd(nc, in_maps, list(range(B)), **kwargs)


def kernel(input_graph, adj, Wq, bq, Wk, bk, Wv, bv, beta):
    in_maps = make_in_maps(input_graph, adj, Wq, bq, Wk, bk, Wv, bv, beta)
    res = run_hw(in_maps)
    return np.stack([res.results[i]["out"] for i in range(B)], axis=0).astype(np.float32)



# revision 7
# speedup vs baseline: 1.0901x; 1.0901x over previous
"""Masked multi-head attention (sparse_attention) Trainium2 Bass kernel.

Data-parallel over batch: B=8 batch elements, one per NeuronCore.
Per-core computation for batch element b (all shapes hardcoded):
  x [1024,768], adj [1024,1024], Wq/Wk/Wv [768,768], bq/bk/bv [768], beta []
  q = x@Wq+bq; k = x@Wk+bk; v = x@Wv+bv      (12 heads of 64)
  S = q k^T / 8 + beta*adj ; masked where adj<=0 ; P = softmax(S)
  out = P v  -> [1024, 768]

v2: single fused software pipeline.  The baseline ran proj -> mask ->
attention -> finalize as serial phases; the ACT engine (which owns the
12.6M-element exp, ~92us at 1 elem/cycle/lane) only worked during the
attention phase, making it the wall.  Here attention for head pair c
starts as soon as qt[c]/kt[c] exist: pair 0's exps issue at ~t=25us and
ACT stays busy for the whole kernel.  V-projection, late QK chunks,
mask chunks and finalize/out-DMA are dribbled into the attention stream
to keep PE/DVE/DMA busy under the ACT roofline.

Per-core dataflow (per head pair c = heads 2c, 2c+1):
  X^T via PE transposes; Q^T,K^T,V = bf16 matmuls (W cast in-flight by
  SWDGE DMA).  m^T[k] = (adjT>0)*exp(beta*adjT) from per-k adj column
  strips.  S^T-pair = K_h @ Q_h^T (two row-tiled K=64 matmuls, emitted
  adjacent for tile concurrency); P^T = m^T * exp(S^T/8) (ACT+DVE);
  out^T[65,512] = [V_h|1]^T @ P^T (ones column = softmax row-sums);
  PE-transpose back per q-chunk, scale by 1/rowsum, DMA out per pair.
"""

import sys

import numpy as np

try:
    import concourse.bass as bass
except ImportError:  # container default location
    sys.path.insert(0, "/opt/trn_rl_repo")
    import concourse.bass as bass

from contextlib import ExitStack

import concourse.bacc as bacc
import concourse.mybir as mybir
import concourse.tile as tile
from concourse.bass_utils import run_bass_kernel_spmd
from concourse.masks import make_identity

B, N, D, H = 8, 1024, 768, 12
HD = 64
P = 128
NT = N // P  # 8 row chunks
DT = D // P  # 6 feature chunks
NH = 512  # free-dim tile for matmuls
HD1 = HD + 1  # head dim + ones column
NPAIR = H // 2  # 6 head pairs

F32 = mybir.dt.float32
BF16 = mybir.dt.bfloat16
AF = mybir.ActivationFunctionType
ALU = mybir.AluOpType


def _emit(tc, ctx, x_d, adj_d, w_d, b_d, beta_d, out_d):
    nc = tc.nc

    const = ctx.enter_context(tc.tile_pool(name="const", bufs=1))
    ident = const.tile([P, P], F32, tag="ident")
    make_identity(nc, ident)

    def bcast(ap, n_part):
        return bass.AP(tensor=ap.tensor, offset=ap.offset, ap=[[0, n_part]] + list(ap.ap))

    beta_sb = const.tile([P, 1], F32, tag="beta")
    nc.gpsimd.dma_start(out=beta_sb, in_=bcast(beta_d[0], P))
    bq_sb = const.tile([P, DT], F32, tag="bq")
    nc.gpsimd.dma_start(out=bq_sb, in_=b_d["bq"].rearrange("(c p) -> p c", p=P))
    bk_sb = const.tile([P, DT], F32, tag="bk")
    nc.gpsimd.dma_start(out=bk_sb, in_=b_d["bk"].rearrange("(c p) -> p c", p=P))
    bv_bc = const.tile([P, D], F32, tag="bv")
    nc.gpsimd.dma_start(out=bv_bc, in_=bcast(b_d["bv"], P))

    # ---- persistent tensors ----
    pers = ctx.enter_context(tc.tile_pool(name="pers", bufs=1))
    w_sb = {}
    for wname in ("wq", "wk", "wv"):
        w_sb[wname] = [
            pers.tile([P, D], BF16, tag=f"{wname}{c}", name=f"{wname}{c}")
            for c in range(DT)
        ]
    xt = [pers.tile([P, N], BF16, tag=f"xt{c}", name=f"xt{c}") for c in range(DT)]
    qt = [pers.tile([P, N], BF16, tag=f"qt{c}", name=f"qt{c}") for c in range(DT)]
    kt = [pers.tile([P, N], BF16, tag=f"kt{c}", name=f"kt{c}") for c in range(DT)]
    v_sb = [pers.tile([P, H * HD1], BF16, tag=f"v{i}", name=f"v{i}") for i in range(NT)]
    m_sb = [pers.tile([P, N], BF16, tag=f"m{k}", name=f"m{k}") for k in range(NT)]
    out_sb = [pers.tile([P, D], F32, tag=f"os{i}", name=f"os{i}") for i in range(NT)]

    # ---- input DMAs (issue everything up front; engines overlap) ----
    # adjp outlives xp: create it first so closing xp pops the alloc stack.
    adjp = ctx.enter_context(tc.tile_pool(name="adjp", bufs=1))
    es_x = ExitStack()
    xp = es_x.enter_context(tc.tile_pool(name="xp", bufs=1))
    x_sb = [xp.tile([P, D], F32, tag=f"x{i}", name=f"x{i}") for i in range(NT)]
    for i in range(NT):
        nc.sync.dma_start(out=x_sb[i], in_=x_d[i * P:(i + 1) * P, :])

    # W: f32 -> bf16 cast in-flight (SWDGE casting DMA); q,k first.
    for wname in ("wq", "wk"):
        for c in range(DT):
            nc.gpsimd.dma_start(
                out=w_sb[wname][c], in_=w_d[wname][c * P:(c + 1) * P, :]
            )

    # adj column strips: strip k holds adj[:, kP:(k+1)P] as [p, (j c)] with
    # strip[p, j*128+c] = adj[j*128+p, k*128+c]  (ring, freed as masks drain)
    adj_strip = [None] * NT

    def emit_adj_dma(k):
        adj_strip[k] = adjp.tile([P, N], F32, tag=f"as{k % 4}", name=f"as{k}", bufs=1)
        nc.sync.dma_start(
            out=adj_strip[k].rearrange("p (j c) -> p j c", c=P),
            in_=adj_d[:, k * P:(k + 1) * P].rearrange("(j p) c -> p j c", p=P),
        )

    for k in range(4):
        emit_adj_dma(k)

    for c in range(DT):
        nc.gpsimd.dma_start(out=w_sb["wv"][c], in_=w_d["wv"][c * P:(c + 1) * P, :])

    # ---- psum pools ----
    work = ctx.enter_context(tc.tile_pool(name="work", space="PSUM", bufs=2))
    pss = ctx.enter_context(tc.tile_pool(name="pss", space="PSUM", bufs=2))
    pso = ctx.enter_context(tc.tile_pool(name="pso", space="PSUM", bufs=2))

    # ---- X^T: per c, two groups of 4 PE transposes + one ACT evac ----
    for c in range(DT):
        for g in range(2):
            tp = work.tile([P, NH], F32, tag="work", name="tp")
            for bb in range(4):
                i = g * 4 + bb
                nc.tensor.transpose(
                    tp[:, bb * P:(bb + 1) * P], x_sb[i][:, c * P:(c + 1) * P], ident
                )
            nc.scalar.copy(xt[c][:, g * NH:(g + 1) * NH], tp)
    es_x.close()

    # ---- masks: m^T[k] = (adjT>0)*exp(beta*adjT) ----
    etm = ctx.enter_context(tc.tile_pool(name="etm", bufs=2))

    def emit_mask(k):
        s3 = adj_strip[k].rearrange("p (j c) -> p j c", c=P)
        for g in range(2):
            tp = work.tile([P, NH], F32, tag="work", name="mtp")
            for bb in range(4):
                j = g * 4 + bb
                nc.tensor.transpose(tp[:, bb * P:(bb + 1) * P], s3[:, j, :], ident)
            e_m = etm.tile([P, NH], BF16, tag="em", name="em")
            nc.scalar.activation(e_m, tp, AF.Exp, scale=beta_sb[:, 0:1])
            nc.vector.scalar_tensor_tensor(
                out=m_sb[k][:, g * NH:(g + 1) * NH],
                in0=tp,
                scalar=0.0,
                in1=e_m,
                op0=ALU.is_gt,
                op1=ALU.mult,
            )

    def emit_qk_chunk(c, wname, dst, bias_sb, qh):
        mm = work.tile([P, NH], F32, tag="work", name="mm")
        for kc in range(DT):
            nc.tensor.matmul(
                mm,
                lhsT=w_sb[wname][kc][:, c * P:(c + 1) * P],
                rhs=xt[kc][:, qh * NH:(qh + 1) * NH],
                start=(kc == 0),
                stop=(kc == DT - 1),
            )
        nc.vector.tensor_scalar_add(
            dst[c][:, qh * NH:(qh + 1) * NH], mm, bias_sb[:, c:c + 1]
        )

    def emit_v(i):
        for s, w in ((0, NH), (NH, D - NH)):
            mm = work.tile([P, NH], F32, tag="work", name="vmm")
            for kc in range(DT):
                nc.tensor.matmul(
                    mm[:, 0:w],
                    lhsT=xt[kc][:, i * P:(i + 1) * P],
                    rhs=w_sb["wv"][kc][:, s:s + w],
                    start=(kc == 0),
                    stop=(kc == DT - 1),
                )
            nh = w // HD
            h0 = s // HD
            dst3 = v_sb[i].rearrange("p (h j) -> p h j", j=HD1)[:, h0:h0 + nh, 0:HD]
            src3 = mm[:, 0:w].rearrange("p (h j) -> p h j", j=HD)
            bias3 = bv_bc[:, s:s + w].rearrange("p (h j) -> p h j", j=HD)
            nc.vector.tensor_add(dst3, src3, bias3)
        ones3 = v_sb[i].rearrange("p (h j) -> p h j", j=HD1)[:, :, HD:HD1]
        nc.vector.memset(ones3, 1.0)

    # masks 0,1 + first two QK chunks before the attention pipeline starts
    emit_mask(0)
    emit_mask(1)
    for c in (0, 1):
        for wname, dst, bias_sb in (("wq", qt, bq_sb), ("wk", kt, bk_sb)):
            for qh in range(2):
                emit_qk_chunk(c, wname, dst, bias_sb, qh)

    # ---- attention pipeline over head pairs ----
    etq = ctx.enter_context(tc.tile_pool(name="etq", bufs=3))
    pp = ctx.enter_context(tc.tile_pool(name="pp", bufs=1))
    otp = ctx.enter_context(tc.tile_pool(name="otp", bufs=2))
    fin = ctx.enter_context(tc.tile_pool(name="fin", bufs=2))

    p_gen = {}  # pair -> [sub][k] tile handles
    ot_sb = [None] * H

    def emit_s_step(c, k):
        """S + exp + mask-mul for pair c, key chunk k."""
        if k == 0:
            p_gen[c] = [[None] * NT for _ in range(2)]
        p_tiles = p_gen[c]
        sps = [pss.tile([P, N], F32, tag="s", name=f"s{sub}") for sub in range(2)]
        # adjacent different-row-group matmuls for tile concurrency
        for qh in range(2):
            for sub in range(2):
                r0 = sub * HD
                nc.tensor.matmul(
                    sps[sub][:, qh * NH:(qh + 1) * NH],
                    lhsT=kt[c][r0:r0 + HD, k * P:(k + 1) * P],
                    rhs=qt[c][r0:r0 + HD, qh * NH:(qh + 1) * NH],
                    start=True,
                    stop=True,
                    tile_position=(r0, 0),
                )
        for sub in range(2):
            e = etq.tile([P, N], BF16, tag="et", name="et")
            nc.scalar.activation(e, sps[sub], AF.Exp, scale=0.125)
            p_tiles[sub][k] = pp.tile(
                [P, N], BF16, tag=f"p{sub}_{k}", name=f"p{sub}_{k}", bufs=1
            )
            nc.vector.tensor_mul(p_tiles[sub][k], e, m_sb[k])

    def emit_pv_piece(c, piece):
        """One (sub, qh) quarter of PV for pair c: 8 accumulating matmuls."""
        sub, qh = piece // 2, piece % 2
        h = 2 * c + sub
        if qh == 0:
            ot_sb[h] = otp.tile([HD1, N], F32, tag=f"ot{sub}", name=f"ot{h}")
        p_tiles = p_gen[c]
        ops = pso.tile([HD1, NH], F32, tag="ov", name="ov")
        for k in range(NT):
            nc.tensor.matmul(
                ops,
                lhsT=v_sb[k][:, h * HD1:(h + 1) * HD1],
                rhs=p_tiles[sub][k][:, qh * NH:(qh + 1) * NH],
                start=(k == 0),
                stop=(k == NT - 1),
            )
        nc.vector.tensor_copy(ot_sb[h][:, qh * NH:(qh + 1) * NH], ops)
        if piece == 3:
            del p_gen[c]

    def emit_fin_qc(c, qc):
        # transpose the pair's out^T for q-chunk qc, scale by 1/rowsum, DMA out
        fp = work.tile([P, NH], F32, tag="work", name="fp")
        for sub in range(2):
            nc.tensor.transpose(
                fp[:, sub * HD1:(sub + 1) * HD1],
                ot_sb[2 * c + sub][:, qc * P:(qc + 1) * P],
                ident[0:HD1, 0:HD1],
            )
        fp3 = fp[:, 0:2 * HD1].rearrange("p (h j) -> p h j", j=HD1)
        rec = fin.tile([P, 2], F32, tag="rec", name="rec")
        nc.vector.reciprocal(rec, fp3[:, :, HD:HD1].squeeze(-1))
        rec_b = bass.AP(
            tensor=rec.tensor, offset=rec.offset, ap=list(rec.ap) + [[0, HD]]
        )
        out3 = out_sb[qc].rearrange("p (h j) -> p h j", j=HD)
        nc.vector.tensor_mul(out3[:, 2 * c:2 * c + 2, :], fp3[:, :, 0:HD], rec_b)
        nc.sync.dma_start(
            out=out_d[qc * P:(qc + 1) * P, c * P:(c + 1) * P],
            in_=out_sb[qc][:, c * P:(c + 1) * P],
        )

    # Steady-state weave per block c:
    #   k step:  S(c,k), then
    #     c==0:  masks k+2, adj dma k+4, V chunk k
    #     c>=1:  k<4: PV piece k of pair c-1;  k>=4: finalize q-chunks of c-1
    #            plus one QK(c+1) projection unit at odd k
    for c in range(NPAIR):
        for k in range(NT):
            emit_s_step(c, k)
            if c == 0:
                if k + 4 < NT:
                    emit_adj_dma(k + 4)
                if k + 2 < NT:
                    emit_mask(k + 2)
                emit_v(k)
            else:
                if k < 4:
                    emit_pv_piece(c - 1, k)
                else:
                    emit_fin_qc(c - 1, 2 * (k - 4))
                    emit_fin_qc(c - 1, 2 * (k - 4) + 1)
                if k % 2 == 1 and c + 1 < DT:
                    wname, dst, bias_sb = (
                        ("wq", qt, bq_sb) if k < 4 else ("wk", kt, bk_sb)
                    )
                    emit_qk_chunk(c + 1, wname, dst, bias_sb, (k // 2) % 2)
    for piece in range(4):
        emit_pv_piece(NPAIR - 1, piece)
    for qc in range(NT):
        emit_fin_qc(NPAIR - 1, qc)


def build_nc():
    nc = bacc.Bacc("TRN2", target_bir_lowering=False, debug=False, num_devices=B)
    x_d = nc.dram_tensor("x", [N, D], F32, kind="ExternalInput").ap()
    adj_d = nc.dram_tensor("adj", [N, N], F32, kind="ExternalInput").ap()
    w_d = {
        "wq": nc.dram_tensor("wq", [D, D], F32, kind="ExternalInput").ap(),
        "wk": nc.dram_tensor("wk", [D, D], F32, kind="ExternalInput").ap(),
        "wv": nc.dram_tensor("wv", [D, D], F32, kind="ExternalInput").ap(),
    }
    b_d = {
        "bq": nc.dram_tensor("bq", [D], F32, kind="ExternalInput").ap(),
        "bk": nc.dram_tensor("bk", [D], F32, kind="ExternalInput").ap(),
        "bv": nc.dram_tensor("bv", [D], F32, kind="ExternalInput").ap(),
    }
    beta_d = nc.dram_tensor("beta", [1, 1], F32, kind="ExternalInput").ap()
    out_d = nc.dram_tensor("out", [N, D], F32, kind="ExternalOutput").ap()
    with tile.TileContext(nc) as tc, ExitStack() as ctx:
        _emit(tc, ctx, x_d, adj_d, w_d, b_d, beta_d, out_d)
    nc.compile()
    return nc


_CACHE = {}


def _get_nc():
    if "nc" not in _CACHE:
        _CACHE["nc"] = build_nc()
    return _CACHE["nc"]


def make_in_maps(input_graph, adj, Wq, bq, Wk, bk, Wv, bv, beta):
    f = lambda a: np.ascontiguousarray(np.asarray(a), dtype=np.float32)
    wq, wk, wv = f(Wq), f(Wk), f(Wv)
    bqa, bka, bva = f(bq), f(bk), f(bv)
    beta_a = f(beta).reshape(1, 1)
    ig, ad = f(input_graph), f(adj)
    return [
        {
            "x": ig[b], "adj": ad[b],
            "wq": wq, "wk": wk, "wv": wv,
            "bq": bqa, "bk": bka, "bv": bva,
            "beta": beta_a,
        }
        for b in range(B)
    ]


def run_hw(in_maps, **kwargs):
    nc = _get_nc()
    return run_bass_kernel_spmd(nc, in_maps, list(range(B)), **kwargs)


def kernel(input_graph, adj, Wq, bq, Wk, bk, Wv, bv, beta):
    in_maps = make_in_maps(input_graph, adj, Wq, bq, Wk, bk, Wv, bv, beta)
    res = run_hw(in_maps)
    return np.stack([res.results[i]["out"] for i in range(B)], axis=0).astype(np.float32)
